# revision 17
# baseline (speedup 1.0000x reference)
"""Trainium2 Bass kernel for nn_Depth3DGridGen.

Computes, for depth (4,1024,2048,1) and transformation (4,4,4):
    g = unit-sphere grid (H,W,4)   [constant]
    p = (g[:,:,:3]*depth, 1) @ T[b]          -> (x,y,z,_)
    r = sqrt(x^2+y^2+z^2) + 1e-4
    theta = arccos(z/r)/(pi/2) - 1 = -(2/pi)*atan(z / sqrt(q + 2e-4*s + 1e-8))
    phi   = atan2(y,x)/pi          = atan(y/x)/pi + (x<0)*sign(y)
    out[..., 0] = phi, out[..., 1] = theta    (4,1024,2048,2)

where q = x^2+y^2, s = sqrt(x^2+y^2+z^2).

Sharding: 8 cores; core c handles batch c//2, row-half c%2 (512 rows).
The grid is separable: g = (sin(th_i)cos(ph_w), sin(th_i)sin(ph_w), cos(th_i)),
so a_j[i,w] = sin(th_i)*A_j[w] + cos(th_i)*T[2,j] is a K=2 matmul computed on
the tensor engine; x = d*a_0 + T[3,0] etc. are fused custom DVE ops.
"""

import os
import sys

import numpy as np

for _p in ("/opt/trn_rl_repo", "/root/.axon_site/_ro/trn_rl_repo"):
    if os.path.isdir(_p) and _p not in sys.path:
        sys.path.insert(0, _p)
        break

from contextlib import ExitStack

import concourse.bass as bass
import concourse.tile as tile
from concourse import bacc, mybir
from concourse import dve_ops
from concourse.bass_utils import run_bass_kernel_spmd
from concourse.dve_spec import Spec, Src0, Src1, C0, Zero, sq, lower, _has_src1
from concourse.dve_uop import DveOpSpec
from concourse.tile import add_dep_helper

F32 = mybir.dt.float32
F32R = mybir.dt.float32r
BS, HEIGHT, WIDTH = 4, 1024, 2048
NCORES = 8
ROWS_PER_CORE = BS * HEIGHT // NCORES  # 512
P = 128
RUNITS = ROWS_PER_CORE // P  # 4 row-tiles
FD = 1024  # free-dim width of a work unit
WCHUNKS = WIDTH // FD  # 2
UNITS = RUNITS * WCHUNKS  # 8 work units of (128, FD)
GROUP = 2  # work units per ACT table-set phase group
AFT = mybir.ActivationFunctionType


# --------------------------------------------------------------------------- #
# Custom DVE ops
# --------------------------------------------------------------------------- #
def _register_op(name: str, spec: Spec):
    for op in dve_ops.OPS:
        if op.name == name:
            return op
    row = dve_ops._CUSTOM_DVE_ROW_BASE + len(dve_ops.OPS)
    assert row < 0x20, "custom DVE opcode rows exhausted"
    shas = {}
    for ver in ("v3", "v4"):
        s = DveOpSpec(
            name=name, opcode=row, uops=lower(spec, ver=ver), rd1_en=_has_src1(spec)
        )
        shas[ver] = s.sha(ver)
    op = dve_ops.DveOp(name, spec, subdim=False, uops_sha=shas)
    dve_ops.OPS.append(op)
    dve_ops.CUSTOM_DVE_SPECS[name] = spec
    dve_ops._SUB_OPCODE_FOR_NAME[name] = row
    return op


# out = in0*in1 + s0   (x = d * a_j + T[3,j])
FMAC = _register_op(
    "FMAC_DGG",
    Spec(
        body=Src0 * Src1 + C0,
        reference=lambda in0, in1, s0, s1, imm2: in0.astype(np.float32) * in1 + s0,
    ),
)
# out = in0^2 + in1^2  (q = x^2 + y^2)
SQ2 = _register_op(
    "SQ2_DGG",
    Spec(
        body=sq(Src0) + sq(Src1),
        reference=lambda in0, in1, s0, s1, imm2: (
            in0.astype(np.float32) * in0 + in1.astype(np.float32) * in1
        ),
    ),
)
# out = in0^2 + in1    (s2 = z^2 + q)
SQADD = _register_op(
    "SQADD_DGG",
    Spec(
        body=sq(Src0) + Src1,
        reference=lambda in0, in1, s0, s1, imm2: in0.astype(np.float32) * in0 + in1,
    ),
)
# out = (in0 < 0) * sign(in1)   (atan2 quadrant correction, in units of pi)
CORR = _register_op(
    "CORR_DGG",
    Spec(
        body=(Src0 < Zero) * ((Src1 > Zero) - (Src1 < Zero)),
        reference=lambda in0, in1, s0, s1, imm2: (
            (in0 < 0).astype(np.float32) * np.sign(in1).astype(np.float32)
        ),
    ),
)

from concourse.dve_ops import AFFINE_THEN_ADD, RECIPROCAL_APPROX_FAST  # noqa: E402


# --------------------------------------------------------------------------- #
# Host-side constants (the grid is a constant of the problem)
# --------------------------------------------------------------------------- #
def _grid_vectors():
    # matches reference _make_grid3d float32 computation exactly
    gx = np.arange(-1.0, 1.0, 2.0 / HEIGHT).astype(np.float32)  # (H,)
    gy = np.arange(-1.0, 1.0, 2.0 / WIDTH).astype(np.float32)  # (W,)
    th = gx * (np.pi / 2) + np.pi / 2
    ph = gy * np.pi
    return (
        np.sin(th).astype(np.float32),
        np.cos(th).astype(np.float32),
        np.cos(ph).astype(np.float32),
        np.sin(ph).astype(np.float32),
    )


_STH, _CTH, _CPH, _SPH = _grid_vectors()


# --------------------------------------------------------------------------- #
# Bass program (input-shape independent of data; built once)
# --------------------------------------------------------------------------- #
_PROGRAM = None


def _build_program():
    nc = bacc.Bacc(
        "TRN2",
        target_bir_lowering=False,
        debug=False,
        enable_asserts=False,
        num_devices=NCORES,
    )
    d_t = nc.dram_tensor("d_in", [ROWS_PER_CORE, WIDTH], F32, kind="ExternalInput")
    lhsT_t = nc.dram_tensor(
        "lhsT_in", [2, 3, ROWS_PER_CORE], F32R, kind="ExternalInput"
    )
    rhs_t = nc.dram_tensor("rhs_in", [2, 3, WIDTH], F32R, kind="ExternalInput")
    scal_t = nc.dram_tensor("scal_in", [P, 4], F32, kind="ExternalInput")
    out_t = nc.dram_tensor(
        "out", [ROWS_PER_CORE, 2 * WIDTH], F32, kind="ExternalOutput"
    )
    d_ap = d_t.ap()
    lhsT_ap = lhsT_t.ap()
    rhs_ap = rhs_t.ap()
    scal_ap = scal_t.ap()
    out_ap = out_t.ap()

    units = [(ru, wc) for ru in range(RUNITS) for wc in range(WCHUNKS)]

    with ExitStack() as ctx:
        tc = ctx.enter_context(tile.TileContext(nc))
        consts = ctx.enter_context(tc.tile_pool(name="consts", bufs=1))
        dpool = ctx.enter_context(tc.tile_pool(name="dp", bufs=2))
        apsum = ctx.enter_context(tc.tile_pool(name="aps", bufs=4, space="PSUM"))
        work = ctx.enter_context(tc.tile_pool(name="work", bufs=2))
        workl = ctx.enter_context(tc.tile_pool(name="workl", bufs=3))
        live = ctx.enter_context(tc.tile_pool(name="live", bufs=4))
        outp = ctx.enter_context(tc.tile_pool(name="outp", bufs=2))

        lhsT_sb = consts.tile([2, 3 * ROWS_PER_CORE], F32R)
        rhs_sb = consts.tile([2, 3 * WIDTH], F32R)
        scal_sb = consts.tile([P, 4], F32)
        nc.sync.dma_start(out=lhsT_sb[:], in_=lhsT_ap.rearrange("a b c -> a (b c)"))
        nc.sync.dma_start(out=rhs_sb[:], in_=rhs_ap.rearrange("a b c -> a (b c)"))
        nc.sync.dma_start(out=scal_sb[:], in_=scal_ap)
        t30 = scal_sb[:, 0:1]
        t31 = scal_sb[:, 1:2]
        t32 = scal_sb[:, 2:3]

        n_groups = UNITS // GROUP
        # ACT instructions are chained batch-by-batch (same activation table
        # set within a batch) so the scheduler never interleaves table sets.
        act_batches = []

        def act_batch(insts):
            if act_batches and insts:
                prev_last = act_batches[-1][-1]
                for i in insts:
                    add_dep_helper(i.ins, prev_last.ins, sync=False, reason="act order")
            if insts:
                act_batches.append(insts)

        for g in range(n_groups):
            gunits = units[g * GROUP : (g + 1) * GROUP]
            lives = {}
            mids = {}

            # ---- phase 1a: DVE chain through s2 / rx / yx / corr ----
            for ru, wc in gunits:
                dtile = dpool.tile([P, FD], F32, tag="d")
                nc.sync.dma_start(
                    out=dtile[:],
                    in_=d_ap[ru * P : (ru + 1) * P, wc * FD : (wc + 1) * FD],
                )

                a = []
                for j in range(3):
                    aj = apsum.tile([P, FD], F32, tag="aps")
                    for n in range(FD // 512):
                        w0 = j * WIDTH + wc * FD + n * 512
                        nc.tensor.matmul(
                            aj[:, n * 512 : (n + 1) * 512],
                            lhsT_sb[
                                0:2,
                                j * ROWS_PER_CORE + ru * P : j * ROWS_PER_CORE
                                + (ru + 1) * P,
                            ],
                            rhs_sb[0:2, w0 : w0 + 512],
                            start=True,
                            stop=True,
                        )
                    a.append(aj)

                x = work.tile([P, FD], F32, tag="x")
                y = work.tile([P, FD], F32, tag="y")
                z = workl.tile([P, FD], F32, tag="z")
                nc.vector._custom_dve(FMAC, out=x[:], in0=dtile[:], in1=a[0][:], s0=t30)
                nc.vector._custom_dve(FMAC, out=y[:], in0=dtile[:], in1=a[1][:], s0=t31)
                nc.vector._custom_dve(FMAC, out=z[:], in0=dtile[:], in1=a[2][:], s0=t32)

                q = workl.tile([P, FD], F32, tag="q")
                nc.vector._custom_dve(SQ2, out=q[:], in0=x[:], in1=y[:])

                rx = work.tile([P, FD], F32, tag="rx")
                nc.vector.reciprocal_approx_fast(out=rx[:], in_=x[:])

                corr = live.tile([P, FD], F32, tag="corr")
                nc.vector._custom_dve(CORR, out=corr[:], in0=x[:], in1=y[:])

                yx = live.tile([P, FD], F32, tag="yx")
                nc.vector.tensor_mul(yx[:], y[:], rx[:])

                s2 = workl.tile([P, FD], F32, tag="s2")
                nc.vector._custom_dve(SQADD, out=s2[:], in0=z[:], in1=q[:])
                mids[(ru, wc)] = (z, q, s2)
                lives[(ru, wc)] = (yx, corr)

            # ---- 1b: s = sqrt(s2)  [sqrt table set] ----
            batch = []
            for u in gunits:
                z, q, s2 = mids[u]
                s = work.tile([P, FD], F32, tag="s")
                batch.append(nc.scalar.activation(s[:], s2[:], AFT.Sqrt))
                mids[u] = (z, q, s)
            act_batch(batch)

            # ---- 1c: den = (s*2e-4 + 1e-8) + q  [DVE] ----
            for u in gunits:
                z, q, s = mids[u]
                den = work.tile([P, FD], F32, tag="den")
                nc.vector._custom_dve(
                    AFFINE_THEN_ADD,
                    out=den[:], in0=s[:], in1=q[:], s0=2.0e-4, s1=1.0e-8,
                )
                mids[u] = (z, den)

            # ---- 1d: lden = ln(den)  [natural_log set] ----
            batch = []
            for u in gunits:
                z, den = mids[u]
                lden = work.tile([P, FD], F32, tag="lden")
                batch.append(nc.scalar.activation(lden[:], den[:], AFT.Ln))
                mids[u] = (z, lden)
            act_batch(batch)

            # ---- 1e: rs = exp(-0.5*lden)  [exp set] ----
            batch = []
            for u in gunits:
                z, lden = mids[u]
                rs = work.tile([P, FD], F32, tag="rs")
                batch.append(nc.scalar.activation(rs[:], lden[:], AFT.Exp, scale=-0.5))
                mids[u] = (z, rs)
            act_batch(batch)

            # ---- 1f: w = z * rs  [DVE] ----
            for u in gunits:
                z, rs = mids[u]
                w = workl.tile([P, FD], F32, tag="w")
                nc.vector.tensor_mul(w[:], z[:], rs[:])
                lives[u] = (w,) + lives[u]

            # ---- phase 2: arctan set + assembly + store ----
            batch = []
            ots = {}
            ats = {}
            for ru, wc in gunits:
                w, yx, corr = lives[(ru, wc)]
                at = work.tile([P, FD], F32, tag="at")
                batch.append(nc.scalar.activation(at[:], w[:], AFT.Arctan))
                at2 = work.tile([P, FD], F32, tag="at2")
                batch.append(nc.scalar.activation(at2[:], yx[:], AFT.Arctan))
                ats[(ru, wc)] = (at, at2)
            for ru, wc in gunits:
                at, at2 = ats[(ru, wc)]
                _, _, corr = lives[(ru, wc)]
                ot = outp.tile([P, 2 * FD], F32, tag="ot")
                ots[(ru, wc)] = ot
                # theta = -(2/pi)*at, interleaved at out[:,1::2]
                batch.append(
                    nc.scalar.activation(
                        ot[:, 1::2], at[:], AFT.Copy, scale=float(-2.0 / np.pi)
                    )
                )
                # phi = at2/pi + corr, interleaved at out[:,0::2]
                nc.vector._custom_dve(
                    AFFINE_THEN_ADD,
                    out=ot[:, 0::2],
                    in0=at2[:],
                    in1=corr[:],
                    s0=float(1.0 / np.pi),
                    s1=0.0,
                )
            act_batch(batch)
            for ru, wc in gunits:
                nc.sync.dma_start(
                    out=out_ap[
                        ru * P : (ru + 1) * P, wc * 2 * FD : (wc + 1) * 2 * FD
                    ],
                    in_=ots[(ru, wc)][:],
                )

    nc.compile()
    return nc


def _get_program():
    global _PROGRAM
    if _PROGRAM is None:
        _PROGRAM = _build_program()
    return _PROGRAM


# --------------------------------------------------------------------------- #
# Host-side wrapper
# --------------------------------------------------------------------------- #
def _make_in_maps(depth: np.ndarray, transformation: np.ndarray):
    depth = np.ascontiguousarray(np.asarray(depth, dtype=np.float32)).reshape(
        BS, HEIGHT, WIDTH
    )
    tr = np.asarray(transformation, dtype=np.float32)

    cph = _CPH.astype(np.float64)
    sph = _SPH.astype(np.float64)
    in_maps = []
    for c in range(NCORES):
        b, h = divmod(c, NCORES // BS)
        T = tr[b].astype(np.float64)
        rows = slice(h * ROWS_PER_CORE, (h + 1) * ROWS_PER_CORE)
        sth = _STH[rows].astype(np.float64)
        cth = _CTH[rows].astype(np.float64)

        lhsT = np.empty((2, 3, ROWS_PER_CORE), dtype=np.float32)
        rhs = np.empty((2, 3, WIDTH), dtype=np.float32)
        for j in range(3):
            lhsT[0, j, :] = sth
            lhsT[1, j, :] = cth * T[2, j]
            rhs[0, j, :] = T[0, j] * cph + T[1, j] * sph
            rhs[1, j, :] = 1.0
        scal = np.empty((P, 4), dtype=np.float32)
        scal[:, 0] = T[3, 0]
        scal[:, 1] = T[3, 1]
        scal[:, 2] = T[3, 2]
        scal[:, 3] = 0.0

        in_maps.append(
            {
                "d_in": np.ascontiguousarray(depth[b, rows, :]),
                "lhsT_in": lhsT,
                "rhs_in": rhs,
                "scal_in": scal,
            }
        )
    return in_maps


def _ensure_ntff_hook():
    """The agent image's antenv lacks axon_hooks; synthesize it so
    run_bass_kernel_spmd(trace=True) can profile via the axon nrt hook."""
    import types

    try:
        from antenv import axon_hooks  # noqa: F401

        return True
    except ImportError:
        pass
    try:
        from trn_agent_boot.trn_boot import _ntff_profile_via_ctypes

        hook = _ntff_profile_via_ctypes("/opt/axon/libaxon_pjrt.so")
        mod = types.ModuleType("antenv.axon_hooks")
        _state = {"hook": hook}
        mod.set_axon_ntff_profile_hook = lambda h: _state.update(hook=h)
        mod.get_axon_ntff_profile_hook = lambda: _state["hook"]
        sys.modules["antenv.axon_hooks"] = mod
        import antenv

        antenv.axon_hooks = mod
        return True
    except Exception as e:  # pragma: no cover
        print(f"ntff hook unavailable: {e}", file=sys.stderr)
        return False


def run(depth, transformation, trace=False):
    """Returns (output (4,1024,2048,2) float32, exec_time_ns or None)."""
    if trace:
        trace = _ensure_ntff_hook()
    nc = _get_program()
    in_maps = _make_in_maps(depth, transformation)
    res = run_bass_kernel_spmd(nc, in_maps, core_ids=list(range(NCORES)), trace=trace)
    out = np.empty((BS, HEIGHT, WIDTH, 2), dtype=np.float32)
    for c in range(NCORES):
        b, h = divmod(c, NCORES // BS)
        rows = slice(h * ROWS_PER_CORE, (h + 1) * ROWS_PER_CORE)
        out[b, rows] = res.results[c]["out"].reshape(ROWS_PER_CORE, WIDTH, 2)
    return out, res.exec_time_ns


def kernel(depth, transformation):
    out, _ = run(depth, transformation, trace=False)
    return out


# revision 19
# speedup vs baseline: 1.1914x; 1.1914x over previous
"""Trainium2 Bass kernel for nn_Depth3DGridGen.

Computes, for depth (4,1024,2048,1) and transformation (4,4,4):
    g = unit-sphere grid (H,W,4)   [constant]
    p = (g[:,:,:3]*depth, 1) @ T[b]          -> (x,y,z,_)
    r = sqrt(x^2+y^2+z^2) + 1e-4
    theta = arccos(z/r)/(pi/2) - 1 = -(2/pi)*atan(z / sqrt(q + 2e-4*s + 1e-8))
    phi   = atan2(y,x)/pi          = atan(y/x)/pi + (x<0)*sign(y)
    out[..., 0] = phi, out[..., 1] = theta    (4,1024,2048,2)

where q = x^2+y^2, s = sqrt(x^2+y^2+z^2).

Sharding: 8 cores; core c handles batch c//2, row-half c%2 (512 rows).
The grid is separable: g = (sin(th_i)cos(ph_w), sin(th_i)sin(ph_w), cos(th_i)),
so a_j[i,w] = sin(th_i)*A_j[w] + cos(th_i)*T[2,j] is a K=2 matmul computed on
the tensor engine; x = d*a_0 + T[3,0] etc. are fused custom DVE ops.
"""

import os
import sys

import numpy as np

for _p in ("/opt/trn_rl_repo", "/root/.axon_site/_ro/trn_rl_repo"):
    if os.path.isdir(_p) and _p not in sys.path:
        sys.path.insert(0, _p)
        break

from contextlib import ExitStack

import concourse.bass as bass
import concourse.tile as tile
from concourse import bacc, mybir
from concourse import dve_ops
from concourse.bass_utils import run_bass_kernel_spmd
from concourse.dve_spec import Spec, Src0, Src1, C0, Zero, sq, lower, _has_src1
from concourse.dve_uop import DveOpSpec
from concourse.tile import add_dep_helper

F32 = mybir.dt.float32
BF16 = mybir.dt.bfloat16
BS, HEIGHT, WIDTH = 4, 1024, 2048
NCORES = 8
ROWS_PER_CORE = BS * HEIGHT // NCORES  # 512
P = 128
RUNITS = ROWS_PER_CORE // P  # 4 row-tiles
FD = 1024  # free-dim width of a work unit
WCHUNKS = WIDTH // FD  # 2
UNITS = RUNITS * WCHUNKS  # 8 work units of (128, FD)
GROUP = 2  # work units per ACT table-set phase group
AFT = mybir.ActivationFunctionType


# --------------------------------------------------------------------------- #
# Custom DVE ops
# --------------------------------------------------------------------------- #
def _register_op(name: str, spec: Spec):
    for op in dve_ops.OPS:
        if op.name == name:
            return op
    row = dve_ops._CUSTOM_DVE_ROW_BASE + len(dve_ops.OPS)
    assert row < 0x20, "custom DVE opcode rows exhausted"
    shas = {}
    for ver in ("v3", "v4"):
        s = DveOpSpec(
            name=name, opcode=row, uops=lower(spec, ver=ver), rd1_en=_has_src1(spec)
        )
        shas[ver] = s.sha(ver)
    op = dve_ops.DveOp(name, spec, subdim=False, uops_sha=shas)
    dve_ops.OPS.append(op)
    dve_ops.CUSTOM_DVE_SPECS[name] = spec
    dve_ops._SUB_OPCODE_FOR_NAME[name] = row
    return op


# out = in0*in1 + s0   (x = d * a_j + T[3,j])
FMAC = _register_op(
    "FMAC_DGG",
    Spec(
        body=Src0 * Src1 + C0,
        reference=lambda in0, in1, s0, s1, imm2: in0.astype(np.float32) * in1 + s0,
    ),
)
# out = in0^2 + in1^2  (q = x^2 + y^2)
SQ2 = _register_op(
    "SQ2_DGG",
    Spec(
        body=sq(Src0) + sq(Src1),
        reference=lambda in0, in1, s0, s1, imm2: (
            in0.astype(np.float32) * in0 + in1.astype(np.float32) * in1
        ),
    ),
)
# out = in0^2 + in1    (s2 = z^2 + q)
SQADD = _register_op(
    "SQADD_DGG",
    Spec(
        body=sq(Src0) + Src1,
        reference=lambda in0, in1, s0, s1, imm2: in0.astype(np.float32) * in0 + in1,
    ),
)
# out = (in0 < 0) * sign(in1)   (atan2 quadrant correction, in units of pi)
CORR = _register_op(
    "CORR_DGG",
    Spec(
        body=(Src0 < Zero) * ((Src1 > Zero) - (Src1 < Zero)),
        reference=lambda in0, in1, s0, s1, imm2: (
            (in0 < 0).astype(np.float32) * np.sign(in1).astype(np.float32)
        ),
    ),
)

from concourse.dve_ops import AFFINE_THEN_ADD, RECIPROCAL_APPROX_FAST  # noqa: E402


# --------------------------------------------------------------------------- #
# Host-side constants (the grid is a constant of the problem)
# --------------------------------------------------------------------------- #
def _grid_vectors():
    # matches reference _make_grid3d float32 computation exactly
    gx = np.arange(-1.0, 1.0, 2.0 / HEIGHT).astype(np.float32)  # (H,)
    gy = np.arange(-1.0, 1.0, 2.0 / WIDTH).astype(np.float32)  # (W,)
    th = gx * (np.pi / 2) + np.pi / 2
    ph = gy * np.pi
    return (
        np.sin(th).astype(np.float32),
        np.cos(th).astype(np.float32),
        np.cos(ph).astype(np.float32),
        np.sin(ph).astype(np.float32),
    )


_STH, _CTH, _CPH, _SPH = _grid_vectors()


# --------------------------------------------------------------------------- #
# Bass program (input-shape independent of data; built once)
# --------------------------------------------------------------------------- #
_PROGRAM = None


def _build_program():
    nc = bacc.Bacc(
        "TRN2",
        target_bir_lowering=False,
        debug=False,
        enable_asserts=False,
        num_devices=NCORES,
    )
    d_t = nc.dram_tensor("d_in", [ROWS_PER_CORE, WIDTH], F32, kind="ExternalInput")
    lhsT_t = nc.dram_tensor(
        "lhsT_in", [9, 3, ROWS_PER_CORE], BF16, kind="ExternalInput"
    )
    rhs_t = nc.dram_tensor("rhs_in", [9, 3, WIDTH], BF16, kind="ExternalInput")
    scal_t = nc.dram_tensor("scal_in", [P, 4], F32, kind="ExternalInput")
    out_t = nc.dram_tensor(
        "out", [ROWS_PER_CORE, 2 * WIDTH], F32, kind="ExternalOutput"
    )
    d_ap = d_t.ap()
    lhsT_ap = lhsT_t.ap()
    rhs_ap = rhs_t.ap()
    scal_ap = scal_t.ap()
    out_ap = out_t.ap()

    units = [(ru, wc) for ru in range(RUNITS) for wc in range(WCHUNKS)]

    with ExitStack() as ctx:
        tc = ctx.enter_context(tile.TileContext(nc))
        consts = ctx.enter_context(tc.tile_pool(name="consts", bufs=1))
        dpool = ctx.enter_context(tc.tile_pool(name="dp", bufs=2))
        apsum = ctx.enter_context(tc.tile_pool(name="aps", bufs=4, space="PSUM"))
        work = ctx.enter_context(tc.tile_pool(name="work", bufs=2))
        workl = ctx.enter_context(tc.tile_pool(name="workl", bufs=3))
        live = ctx.enter_context(tc.tile_pool(name="live", bufs=4))
        outp = ctx.enter_context(tc.tile_pool(name="outp", bufs=2))

        lhsT_sb = consts.tile([9, 3 * ROWS_PER_CORE], BF16)
        rhs_sb = consts.tile([9, 3 * WIDTH], BF16)
        scal_sb = consts.tile([P, 4], F32)
        nc.sync.dma_start(out=lhsT_sb[:], in_=lhsT_ap.rearrange("a b c -> a (b c)"))
        nc.sync.dma_start(out=rhs_sb[:], in_=rhs_ap.rearrange("a b c -> a (b c)"))
        nc.sync.dma_start(out=scal_sb[:], in_=scal_ap)
        t30 = scal_sb[:, 0:1]
        t31 = scal_sb[:, 1:2]
        t32 = scal_sb[:, 2:3]

        n_groups = UNITS // GROUP
        # ACT instructions are chained batch-by-batch (same activation table
        # set within a batch) so the scheduler never interleaves table sets.
        act_batches = []

        def act_batch(insts):
            if act_batches and insts:
                prev_last = act_batches[-1][-1]
                for i in insts:
                    add_dep_helper(i.ins, prev_last.ins, sync=False, reason="act order")
            if insts:
                act_batches.append(insts)

        for g in range(n_groups):
            gunits = units[g * GROUP : (g + 1) * GROUP]
            lives = {}
            mids = {}

            # ---- phase 1a: DVE chain through s2 / rx / yx / corr ----
            for ru, wc in gunits:
                dtile = dpool.tile([P, FD], F32, tag="d")
                nc.sync.dma_start(
                    out=dtile[:],
                    in_=d_ap[ru * P : (ru + 1) * P, wc * FD : (wc + 1) * FD],
                )

                a = []
                for j in range(3):
                    aj = apsum.tile([P, FD], F32, tag="aps")
                    for n in range(FD // 512):
                        w0 = j * WIDTH + wc * FD + n * 512
                        nc.tensor.matmul(
                            aj[:, n * 512 : (n + 1) * 512],
                            lhsT_sb[
                                0:9,
                                j * ROWS_PER_CORE + ru * P : j * ROWS_PER_CORE
                                + (ru + 1) * P,
                            ],
                            rhs_sb[0:9, w0 : w0 + 512],
                            start=True,
                            stop=True,
                        )
                    a.append(aj)

                x = work.tile([P, FD], F32, tag="x")
                y = work.tile([P, FD], F32, tag="y")
                z = workl.tile([P, FD], F32, tag="z")
                nc.vector._custom_dve(FMAC, out=x[:], in0=dtile[:], in1=a[0][:], s0=t30)
                nc.vector._custom_dve(FMAC, out=y[:], in0=dtile[:], in1=a[1][:], s0=t31)
                nc.vector._custom_dve(FMAC, out=z[:], in0=dtile[:], in1=a[2][:], s0=t32)

                q = workl.tile([P, FD], F32, tag="q")
                nc.vector._custom_dve(SQ2, out=q[:], in0=x[:], in1=y[:])

                rx = work.tile([P, FD], F32, tag="rx")
                nc.vector.reciprocal_approx_fast(out=rx[:], in_=x[:])

                corr = live.tile([P, FD], F32, tag="corr")
                nc.vector._custom_dve(CORR, out=corr[:], in0=x[:], in1=y[:])

                yx = live.tile([P, FD], F32, tag="yx")
                nc.vector.tensor_mul(yx[:], y[:], rx[:])

                s2 = workl.tile([P, FD], F32, tag="s2")
                nc.vector._custom_dve(SQADD, out=s2[:], in0=z[:], in1=q[:])
                mids[(ru, wc)] = (z, q, s2)
                lives[(ru, wc)] = (yx, corr)

            # ---- 1b: s = sqrt(s2)  [sqrt table set] ----
            batch = []
            for u in gunits:
                z, q, s2 = mids[u]
                s = work.tile([P, FD], F32, tag="s")
                batch.append(nc.scalar.activation(s[:], s2[:], AFT.Sqrt))
                mids[u] = (z, q, s)
            act_batch(batch)

            # ---- 1c: den = (s*2e-4 + 1e-8) + q  [DVE] ----
            for u in gunits:
                z, q, s = mids[u]
                den = work.tile([P, FD], F32, tag="den")
                nc.vector._custom_dve(
                    AFFINE_THEN_ADD,
                    out=den[:], in0=s[:], in1=q[:], s0=2.0e-4, s1=1.0e-8,
                )
                mids[u] = (z, den)

            # ---- 1d: lden = ln(den)  [natural_log set] ----
            batch = []
            for u in gunits:
                z, den = mids[u]
                lden = work.tile([P, FD], F32, tag="lden")
                batch.append(nc.scalar.activation(lden[:], den[:], AFT.Ln))
                mids[u] = (z, lden)
            act_batch(batch)

            # ---- 1e: rs = exp(-0.5*lden)  [exp set] ----
            batch = []
            for u in gunits:
                z, lden = mids[u]
                rs = work.tile([P, FD], F32, tag="rs")
                batch.append(nc.scalar.activation(rs[:], lden[:], AFT.Exp, scale=-0.5))
                mids[u] = (z, rs)
            act_batch(batch)

            # ---- 1f: w = z * rs  [DVE] ----
            for u in gunits:
                z, rs = mids[u]
                w = workl.tile([P, FD], F32, tag="w")
                nc.vector.tensor_mul(w[:], z[:], rs[:])
                lives[u] = (w,) + lives[u]

            # ---- phase 2: arctan set + assembly + store ----
            batch = []
            ots = {}
            ats = {}
            for ru, wc in gunits:
                w, yx, corr = lives[(ru, wc)]
                at = work.tile([P, FD], F32, tag="at")
                batch.append(nc.scalar.activation(at[:], w[:], AFT.Arctan))
                at2 = work.tile([P, FD], F32, tag="at2")
                batch.append(nc.scalar.activation(at2[:], yx[:], AFT.Arctan))
                ats[(ru, wc)] = (at, at2)
            for ru, wc in gunits:
                at, at2 = ats[(ru, wc)]
                _, _, corr = lives[(ru, wc)]
                ot = outp.tile([P, 2 * FD], F32, tag="ot")
                ots[(ru, wc)] = ot
                # theta = -(2/pi)*at, interleaved at out[:,1::2]
                batch.append(
                    nc.scalar.activation(
                        ot[:, 1::2], at[:], AFT.Copy, scale=float(-2.0 / np.pi)
                    )
                )
                # phi = at2/pi + corr, interleaved at out[:,0::2]
                nc.vector._custom_dve(
                    AFFINE_THEN_ADD,
                    out=ot[:, 0::2],
                    in0=at2[:],
                    in1=corr[:],
                    s0=float(1.0 / np.pi),
                    s1=0.0,
                )
            act_batch(batch)
            for ru, wc in gunits:
                nc.sync.dma_start(
                    out=out_ap[
                        ru * P : (ru + 1) * P, wc * 2 * FD : (wc + 1) * 2 * FD
                    ],
                    in_=ots[(ru, wc)][:],
                )

    nc.compile()
    return nc


def _get_program():
    global _PROGRAM
    if _PROGRAM is None:
        _PROGRAM = _build_program()
    return _PROGRAM


# --------------------------------------------------------------------------- #
# Host-side wrapper
# --------------------------------------------------------------------------- #
def _split3(v64: np.ndarray):
    """Split float64 values into 3 bf16 components summing to ~24-bit accuracy."""
    import ml_dtypes

    bf = ml_dtypes.bfloat16
    h = v64.astype(bf)
    r1 = v64 - h.astype(np.float64)
    m = r1.astype(bf)
    r2 = r1 - m.astype(np.float64)
    l = r2.astype(bf)
    return h, m, l


def _make_in_maps(depth: np.ndarray, transformation: np.ndarray):
    import ml_dtypes

    bf = ml_dtypes.bfloat16
    depth = np.ascontiguousarray(np.asarray(depth, dtype=np.float32)).reshape(
        BS, HEIGHT, WIDTH
    )
    tr = np.asarray(transformation, dtype=np.float32)

    cph = _CPH.astype(np.float64)
    sph = _SPH.astype(np.float64)
    in_maps = []
    for c in range(NCORES):
        b, h = divmod(c, NCORES // BS)
        T = tr[b].astype(np.float64)
        rows = slice(h * ROWS_PER_CORE, (h + 1) * ROWS_PER_CORE)
        sth = _STH[rows].astype(np.float64)
        cth = _CTH[rows].astype(np.float64)

        # a_j = sth*A_j + (cth*T2j)*1, computed as one K=9 bf16 matmul with
        # 3-way bf16-split operands:
        #   sth*A ~= sh*AH + sh*AM + sm*AH + sh*AL + sm*AM + sl*AH
        #   (cth*T2j)*1 = (ch + cm + cl)*1
        lhsT = np.empty((9, 3, ROWS_PER_CORE), dtype=bf)
        rhs = np.empty((9, 3, WIDTH), dtype=bf)
        for j in range(3):
            sh, sm, sl = _split3(sth)
            ch, cm, cl = _split3(cth * T[2, j])
            A = T[0, j] * cph + T[1, j] * sph
            AH, AM, AL = _split3(A)
            for k, row in enumerate((sh, sh, sm, sh, sm, sl, ch, cm, cl)):
                lhsT[k, j, :] = row
            for k, row in enumerate((AH, AM, AH, AL, AM, AH)):
                rhs[k, j, :] = row
            rhs[6:9, j, :] = bf(1.0)
        scal = np.empty((P, 4), dtype=np.float32)
        scal[:, 0] = T[3, 0]
        scal[:, 1] = T[3, 1]
        scal[:, 2] = T[3, 2]
        scal[:, 3] = 0.0

        in_maps.append(
            {
                "d_in": np.ascontiguousarray(depth[b, rows, :]),
                "lhsT_in": lhsT,
                "rhs_in": rhs,
                "scal_in": scal,
            }
        )
    return in_maps


def _ensure_ntff_hook():
    """The agent image's antenv lacks axon_hooks; synthesize it so
    run_bass_kernel_spmd(trace=True) can profile via the axon nrt hook."""
    import types

    try:
        from antenv import axon_hooks  # noqa: F401

        return True
    except ImportError:
        pass
    try:
        from trn_agent_boot.trn_boot import _ntff_profile_via_ctypes

        hook = _ntff_profile_via_ctypes("/opt/axon/libaxon_pjrt.so")
        mod = types.ModuleType("antenv.axon_hooks")
        _state = {"hook": hook}
        mod.set_axon_ntff_profile_hook = lambda h: _state.update(hook=h)
        mod.get_axon_ntff_profile_hook = lambda: _state["hook"]
        sys.modules["antenv.axon_hooks"] = mod
        import antenv

        antenv.axon_hooks = mod
        return True
    except Exception as e:  # pragma: no cover
        print(f"ntff hook unavailable: {e}", file=sys.stderr)
        return False


def run(depth, transformation, trace=False):
    """Returns (output (4,1024,2048,2) float32, exec_time_ns or None)."""
    if trace:
        trace = _ensure_ntff_hook()
    nc = _get_program()
    in_maps = _make_in_maps(depth, transformation)
    res = run_bass_kernel_spmd(nc, in_maps, core_ids=list(range(NCORES)), trace=trace)
    out = np.empty((BS, HEIGHT, WIDTH, 2), dtype=np.float32)
    for c in range(NCORES):
        b, h = divmod(c, NCORES // BS)
        rows = slice(h * ROWS_PER_CORE, (h + 1) * ROWS_PER_CORE)
        out[b, rows] = res.results[c]["out"].reshape(ROWS_PER_CORE, WIDTH, 2)
    return out, res.exec_time_ns


def kernel(depth, transformation):
    out, _ = run(depth, transformation, trace=False)
    return out
